# revision 37
# baseline (speedup 1.0000x reference)
"""CrissCross (axial) attention on 8 NeuronCores — hand-written Bass/Tile kernel.

Problem (hardcoded from the spec):
  x     [16, 64, 128, 128] f32      Wq,Wk [8, 64]  bq,bk [8]
  Wv    [64, 64]  bv [64]           gamma [1]
  out = gamma * (att_H @ v + att_W @ v) + x   (joint softmax over H+W keys,
                                               H diagonal masked)

Sharding: pure data parallel — batch 16 split 2-per-core across 8 cores,
weights replicated.  Each core runs the same single-core Bass program.

Per-core algorithm (per batch image, everything stays in SBUF):
  * x~ = [bf16(x); ones-row]                                  [65, 16384]
  * Gq = (Wq'^T Wk')^T @ x~   (Gram-fused q/k projection)     [65, 16384]
  * per column w (H family) and per row h (W family):
      - one PE matmul makes e^T[key, query] = x~_s^T Gq_s     [128, 128]
      - vT tile [pos, 64+1] = x~_s^T WvT~ (ones col -> z row;
        gamma and biases folded into the weights)
      - ACT exp evicts PSUM->SBUF bf16; (1-I) mask mult for H family
      - PV matmul  [65, 128] = vT~^T P^T ; row 64 accumulates z
  * PV tiles evict into out_bf [65, 16384]; row 64 assembles z
  * rz = 1/z via chop->reciprocal_approx_fast->replicate (DMA)
  * out = out_bf * rz + x  (x re-read in f32), DMA to HBM
"""

import time

import numpy as np

import jax

import concourse.bass as bass
import concourse.bacc as bacc
import concourse.tile as tile
from concourse import mybir
from concourse.bass2jax import (
    _bass_exec_p,
    install_neuronx_cc_hook,
    partition_id_tensor,
)
from concourse.masks import make_identity

B, C, H, W = 16, 64, 128, 128
HW = H * W
C8 = 8
N_CORES = 8
B2 = B // N_CORES  # batches per core
CA = C + 1  # channel dim augmented with the bias/ones row
F32 = mybir.dt.float32
BF16 = mybir.dt.bfloat16
AF = mybir.ActivationFunctionType
ALU = mybir.AluOpType


def build_nc():
    nc = bacc.Bacc(
        "TRN2", target_bir_lowering=False, debug=False, enable_asserts=True
    )
    x_d = nc.dram_tensor("x", [B2, C, HW], F32, kind="ExternalInput")
    wq_d = nc.dram_tensor("wq", [C8, C], F32, kind="ExternalInput")
    bq_d = nc.dram_tensor("bq", [C8, 1], F32, kind="ExternalInput")
    wk_d = nc.dram_tensor("wk", [C8, C], F32, kind="ExternalInput")
    bk_d = nc.dram_tensor("bk", [C8, 1], F32, kind="ExternalInput")
    wv_d = nc.dram_tensor("wv", [C, C], F32, kind="ExternalInput")
    bv_d = nc.dram_tensor("bv", [C, 1], F32, kind="ExternalInput")
    g_d = nc.dram_tensor("gamma", [1, 1], F32, kind="ExternalInput")
    out_d = nc.dram_tensor("out", [B2, C, HW], F32, kind="ExternalOutput")

    with tile.TileContext(nc) as tc:
        _build_body(nc, tc, x_d, wq_d, bq_d, wk_d, bk_d, wv_d, bv_d, g_d, out_d)
    nc.compile()
    return nc


def _build_body(nc, tc, x_d, wq_d, bq_d, wk_d, bk_d, wv_d, bv_d, g_d, out_d):
    from contextlib import ExitStack

    with ExitStack() as ctx:
        ep = ctx.enter_context  # shorthand

        # ---- persistent SBUF state (bufs=1 pools) ----
        const = ep(tc.tile_pool(name="const", bufs=1))
        big = ep(tc.tile_pool(name="big", bufs=1))

        xaug = big.tile([CA, HW], BF16, tag="xaug")
        gq = big.tile([CA, HW], BF16, tag="gq")
        vt_col = big.tile([128, CA * W], BF16, tag="vtc")  # tile w at [:, 65w:65w+65]
        vt_row = big.tile([128, CA * H], BF16, tag="vtr")
        out_bf = big.tile([CA, HW], BF16, tag="outbf")  # rows 0..63 att, row 64 z

        gt_bf = const.tile([CA, CA], BF16, tag="gt")  # (Wk'^T Wq')^T
        wvtg = const.tile([CA, CA], BF16, tag="wvtg")  # [gamma*Wv^T~ | e64]
        mask = const.tile([128, 128], BF16, tag="mask")  # 1 - I
        ident = const.tile([C, C], BF16, tag="ident")
        z_sq = const.tile([128, 128], BF16, tag="zsq")
        z_f32 = const.tile([128, 128], F32, tag="zf32")
        rz_f32 = const.tile([128, 128], F32, tag="rzf32")
        rz_sq = const.tile([128, 128], BF16, tag="rzsq")

        # ---- scratch pools ----
        wpool = ep(tc.tile_pool(name="wts", bufs=1))
        chunks = ep(tc.tile_pool(name="chunks", bufs=2))
        fchunks = ep(tc.tile_pool(name="fchunks", bufs=3))
        ppool = ep(tc.tile_pool(name="ppool", bufs=6))
        dram = ep(tc.tile_pool(name="dram", bufs=1, space="DRAM"))
        rz_dram = dram.tile([1, HW], BF16, tag="rzd")
        z_dram = dram.tile([1, HW], BF16, tag="zd")
        ps_small = ep(tc.tile_pool(name="ps_small", bufs=3, space="PSUM"))
        ps_t = ep(tc.tile_pool(name="ps_t", bufs=1, space="PSUM"))
        ps_e = ep(tc.tile_pool(name="ps_e", bufs=2, space="PSUM"))

        # =================== setup ===================
        # raw weights to SBUF (f32)
        wq_f = wpool.tile([C8, C], F32, tag="wqf")
        wk_f = wpool.tile([C8, C], F32, tag="wkf")
        wv_f = wpool.tile([C, C], F32, tag="wvf")
        bq_f = wpool.tile([C8, 1], F32, tag="bqf")
        bk_f = wpool.tile([C8, 1], F32, tag="bkf")
        bv_f = wpool.tile([C, 1], F32, tag="bvf")
        g_f = wpool.tile([1, 1], F32, tag="gf")
        nc.sync.dma_start(wq_f[:], wq_d.ap())
        nc.sync.dma_start(wk_f[:], wk_d.ap())
        nc.sync.dma_start(wv_f[:], wv_d.ap())
        nc.sync.dma_start(bq_f[:], bq_d.ap())
        nc.sync.dma_start(bk_f[:], bk_d.ap())
        nc.sync.dma_start(bv_f[:], bv_d.ap())
        nc.sync.dma_start(g_f[:], g_d.ap())

        # augmented q/k weights  Wq' = [Wq | bq]  -> bf16
        wqp = wpool.tile([C8, CA], BF16, tag="wqp")
        wkp = wpool.tile([C8, CA], BF16, tag="wkp")
        nc.vector.tensor_copy(wqp[:, 0:C], wq_f[:])
        nc.vector.tensor_copy(wqp[:, C:CA], bq_f[:])
        nc.vector.tensor_copy(wkp[:, 0:C], wk_f[:])
        nc.vector.tensor_copy(wkp[:, C:CA], bk_f[:])

        # GT = Wq'^T Wk'  (so that GT^T @ x~ contracts correctly)
        gt_ps = ps_small.tile([CA, 512], F32, tag="mp")
        nc.tensor.matmul(gt_ps[:, 0:CA], wqp[:], wkp[:], start=True, stop=True)
        nc.scalar.copy(gt_bf[:], gt_ps[:, 0:CA])

        # identity (bf16) for PE transposes
        make_identity(nc, ident[:])

        # gamma broadcast to [C, 1] f32 via f32 matmul with ones row
        ones_row = wpool.tile([1, C], F32, tag="ones_row")
        nc.vector.memset(ones_row[:], 1.0)
        gcol_ps = ps_small.tile([C, 512], F32, tag="mp")
        nc.tensor.matmul(gcol_ps[:, 0:1], ones_row[:], g_f[:], start=True, stop=True)
        gcol = wpool.tile([C, 1], F32, tag="gcol")
        nc.scalar.copy(gcol[:], gcol_ps[:, 0:1])

        # WvT~g: rows 0..63 = gamma*Wv^T, row 64 = gamma*bv^T, col 64 = e_64
        wv_bf = wpool.tile([C, C], BF16, tag="wvbf")
        nc.vector.tensor_copy(wv_bf[:], wv_f[:])
        wvt_ps = ps_t.tile([C, 512], BF16, tag="mpT")
        nc.tensor.matmul(
            wvt_ps[:, 0:C], wv_bf[:], ident[:], is_transpose=True, start=True, stop=True
        )
        nc.scalar.activation(
            wvtg[0:C, 0:C], wvt_ps[:, 0:C], AF.Copy, scale=gcol[:]
        )
        bv_bf = wpool.tile([C, 1], BF16, tag="bvbf")
        nc.vector.tensor_copy(bv_bf[:], bv_f[:])
        bvt_ps = ps_t.tile([1, 512], BF16, tag="mpT")
        nc.tensor.matmul(
            bvt_ps[:, 0:C],
            bv_bf[:],
            ident[:],
            is_transpose=True,
            start=True,
            stop=True,
        )
        g11 = wpool.tile([1, 1], F32, tag="g11")
        nc.vector.tensor_copy(g11[:], g_f[:])
        nc.scalar.activation(wvtg[C : C + 1, 0:C], bvt_ps[:, 0:C], AF.Copy, scale=g11[:])
        nc.vector.memset(wvtg[0:C, C : C + 1], 0.0)
        nc.vector.memset(wvtg[C : C + 1, C : C + 1], 1.0)

        # mask = 1 - I  (bf16)
        nc.vector.memset(mask[:], 1.0)
        nc.gpsimd.affine_select(
            out=mask[:],
            in_=mask[:],
            compare_op=ALU.not_equal,
            fill=0.0,
            base=0,
            pattern=[[-1, 128]],
            channel_multiplier=1,
        )

        # ones row of x~ (written once; rows 0..63 rewritten per batch)
        nc.vector.memset(xaug[C : C + 1, :], 1.0)

        # =================== per-batch body ===================
        for b in range(B2):
            xa3 = xaug[0:C, :].rearrange("c (h w) -> c h w", w=W)
            ga3 = gq[:, :].rearrange("c (h w) -> c h w", w=W)
            xf3 = xaug[:, :].rearrange("c (h w) -> c h w", w=W)

            # ---- load x, cast to bf16 into xaug rows 0..63 ----
            NL = 2048
            for k in range(HW // NL):
                xc = chunks.tile([C, NL], F32, tag="xc")
                nc.sync.dma_start(xc[:], x_d.ap()[b, :, bass.ts(k, NL)])
                nc.vector.tensor_copy(xaug[0:C, bass.ts(k, NL)], xc[:])

            # ---- Gq = GT^T @ x~ ----
            for k in range(HW // 512):
                gq_ps = ps_small.tile([CA, 512], F32, tag="mp")
                nc.tensor.matmul(
                    gq_ps[:], gt_bf[:], xaug[:, bass.ts(k, 512)], start=True, stop=True
                )
                nc.scalar.copy(gq[:, bass.ts(k, 512)], gq_ps[:])

            # ---- vT tiles (7 per PSUM bank), both families ----
            for fam in range(2):
                vt_sb = vt_col if fam == 0 else vt_row
                for g0 in range(0, 128, 7):
                    gn = min(7, 128 - g0)
                    vt_ps = ps_small.tile([128, 512], F32, tag="mp")
                    for j in range(gn):
                        t = g0 + j
                        xs = xf3[:, :, t] if fam == 0 else xf3[:, t, :]
                        nc.tensor.matmul(
                            vt_ps[:, j * CA : (j + 1) * CA],
                            xs,
                            wvtg[:],
                            start=True,
                            stop=True,
                        )
                    ev = vt_sb[:, g0 * CA : (g0 + gn) * CA]
                    if (g0 // 7) % 2 == (0 if fam == 0 else 1):
                        nc.scalar.copy(ev, vt_ps[:, 0 : gn * CA])
                    else:
                        nc.vector.tensor_copy(ev, vt_ps[:, 0 : gn * CA])

            # ---- main loop: H family (copy) then W family (add) ----
            EG = 8  # energy tiles per PSUM chunk (2 banks)
            PG = 4  # PV tiles per PSUM bank
            for fam in (0, 1):
                vt_sb = vt_col if fam == 0 else vt_row
                for t0 in range(0, 128, EG):
                    e_ps = ps_e.tile([128, EG * 128], F32, tag="eps")
                    for j in range(EG):
                        t = t0 + j
                        xs = xf3[:, :, t] if fam == 0 else xf3[:, t, :]
                        gs = ga3[:, :, t] if fam == 0 else ga3[:, t, :]
                        nc.tensor.matmul(
                            e_ps[:, bass.ts(j, 128)], xs, gs, start=True, stop=True
                        )
                    p_sb = ppool.tile([128, EG * 128], BF16, tag="p")
                    nc.scalar.activation(p_sb[:], e_ps[:], AF.Exp)
                    if fam == 0:
                        # zero the diagonal of each tile: P *= (1 - I)
                        m_b = mask[:].unsqueeze(1).broadcast_to((128, EG, 128))
                        p3 = p_sb[:].rearrange("p (t q) -> p t q", t=EG)
                        nc.vector.tensor_tensor(p3, p3, m_b, op=ALU.mult)
                    for q0 in range(0, EG, PG):
                        pv_ps = ps_small.tile([CA, 512], F32, tag="mp")
                        for j in range(PG):
                            t = t0 + q0 + j
                            nc.tensor.matmul(
                                pv_ps[:, bass.ts(j, 128)],
                                vt_sb[:, t * CA : (t + 1) * CA],
                                p_sb[:, bass.ts(q0 + j, 128)],
                                start=True,
                                stop=True,
                            )
                        if fam == 0:
                            # H family: scatter-copy to out_bf[:, h*W + w]
                            w0 = t0 + q0
                            ob = out_bf[:].rearrange("c (h w) -> c h w", w=W)[
                                :, :, w0 : w0 + PG
                            ]
                            pv3 = pv_ps[:].rearrange("c (t h) -> c h t", t=PG)
                            if (w0 // PG) % 2 == 0:
                                nc.scalar.copy(ob, pv3)
                            else:
                                nc.vector.tensor_copy(ob, pv3)
                        else:
                            # W family: contiguous add into out_bf
                            dst = out_bf[:, (t0 + q0) * W : (t0 + q0 + PG) * W]
                            nc.vector.tensor_tensor(
                                dst, pv_ps[:], dst, op=ALU.add
                            )

            # ---- rz = 1/z  (z row -> DRAM -> square tile -> recip -> DRAM) ----
            nc.sync.dma_start(z_dram[:], out_bf[C : C + 1, :])
            nc.sync.dma_start(
                z_sq[:], z_dram[:].rearrange("o (p q) -> (o p) q", q=128)
            )
            nc.vector.tensor_copy(z_f32[:], z_sq[:])
            nc.vector.reciprocal_approx_fast(rz_f32[:], z_f32[:])
            nc.vector.tensor_copy(rz_sq[:], rz_f32[:])
            nc.sync.dma_start(
                rz_dram[:].rearrange("o (p q) -> (o p) q", q=128), rz_sq[:]
            )

            # ---- final: out = att*rz + x ----
            NF = 1024
            for k in range(HW // NF):
                rzb = fchunks.tile([C, NF], BF16, tag="rzb")
                nc.sync.dma_start(
                    rzb[:], rz_dram[:, bass.ts(k, NF)].broadcast_to((C, NF))
                )
                tb = fchunks.tile([C, NF], BF16, tag="tb")
                nc.vector.tensor_tensor(
                    tb[:], out_bf[0:C, bass.ts(k, NF)], rzb[:], op=ALU.mult
                )
                of = fchunks.tile([C, NF], F32, tag="of")
                if b == 0:
                    # exact f32 x re-read; Pool add overlaps batch 1 compute
                    xf = fchunks.tile([C, NF], F32, tag="xf")
                    nc.sync.dma_start(xf[:], x_d.ap()[b, :, bass.ts(k, NF)])
                    eng = nc.gpsimd if k % 2 == 0 else nc.vector
                    eng.tensor_tensor(of[:], xf[:], tb[:], op=ALU.add)
                else:
                    # tail: last batch -- bf16 x already in SBUF, no DMA;
                    # alternate DVE/Pool so adds stream in parallel
                    eng = nc.vector if k % 2 == 0 else nc.gpsimd
                    eng.tensor_tensor(
                        of[:], xaug[0:C, bass.ts(k, NF)], tb[:], op=ALU.add
                    )
                nc.sync.dma_start(out_d.ap()[b, :, bass.ts(k, NF)], of[:])


# =====================================================================
# host-side wrapper: shard inputs, run the Bass program on 8 cores
# =====================================================================

_cache = {}


def _get_runner():
    if "run" in _cache:
        return _cache["run"]
    from jax.sharding import Mesh, PartitionSpec, NamedSharding
    from jax.experimental.shard_map import shard_map

    nc = build_nc()
    install_neuronx_cc_hook()

    in_names = ["x", "wq", "bq", "wk", "bk", "wv", "bv", "gamma", "out",
                "partition_id"]
    out_names = ["out"]
    out_avals = [jax.core.ShapedArray((B2, C, HW), np.float32)]

    def _body(*args):
        outs = _bass_exec_p.bind(
            *args,
            partition_id_tensor(),
            out_avals=tuple(out_avals),
            in_names=tuple(in_names),
            out_names=tuple(out_names),
            lowering_input_output_aliases=(),
            sim_require_finite=True,
            sim_require_nnan=True,
            nc=nc,
        )
        return outs[0]

    devices = jax.devices()[:N_CORES]
    mesh = Mesh(np.asarray(devices), ("core",))
    spec = NamedSharding(mesh, PartitionSpec("core"))
    P = PartitionSpec("core")
    f = jax.jit(
        shard_map(
            _body,
            mesh=mesh,
            in_specs=(P,) * 9,
            out_specs=P,
            check_rep=False,
        ),
        keep_unused=True,
    )
    _cache["run"] = (f, mesh, spec)
    return _cache["run"]


def _shard_inputs(x, Wq, bq, Wk, bk, Wv, bv, gamma):
    """Concatenated per-core operands (core-major on axis 0)."""
    xs = np.ascontiguousarray(x, np.float32).reshape(N_CORES, B2, C, HW)
    xs = xs.reshape(N_CORES * B2, C, HW)

    def rep(a, shape):
        a = np.ascontiguousarray(a, np.float32).reshape(shape)
        return np.concatenate([a] * N_CORES, axis=0)

    return [
        xs,
        rep(Wq, (C8, C)),
        rep(np.asarray(bq).reshape(C8, 1), (C8, 1)),
        rep(Wk, (C8, C)),
        rep(np.asarray(bk).reshape(C8, 1), (C8, 1)),
        rep(Wv, (C, C)),
        rep(np.asarray(bv).reshape(C, 1), (C, 1)),
        rep(np.asarray(gamma).reshape(1, 1), (1, 1)),
        np.zeros((N_CORES * B2, C, HW), np.float32),
    ]


def kernel(x, Wq, bq, Wk, bk, Wv, bv, gamma):
    f, mesh, spec = _get_runner()
    ops = _shard_inputs(x, Wq, bq, Wk, bk, Wv, bv, gamma)
    out = f(*ops)
    out = np.asarray(out).reshape(B, C, H, W).astype(np.float32)
    return out


# revision 38
# speedup vs baseline: 1.4068x; 1.4068x over previous
"""CrissCross (axial) attention on 8 NeuronCores — hand-written Bass/Tile kernel.

Problem (hardcoded from the spec):
  x     [16, 64, 128, 128] f32      Wq,Wk [8, 64]  bq,bk [8]
  Wv    [64, 64]  bv [64]           gamma [1]
  out = gamma * (att_H @ v + att_W @ v) + x   (joint softmax over H+W keys,
                                               H diagonal masked)

Sharding: pure data parallel — batch 16 split 2-per-core across 8 cores,
weights replicated.  Each core runs the same single-core Bass program.

Per-core algorithm (per batch image, everything stays in SBUF):
  * x~ = [bf16(x); ones-row]                                  [65, 16384]
  * Gq = (Wq'^T Wk')^T @ x~   (Gram-fused q/k projection)     [65, 16384]
  * per column w (H family) and per row h (W family):
      - one PE matmul makes e^T[key, query] = x~_s^T Gq_s     [128, 128]
      - vT tile [pos, 64+1] = x~_s^T WvT~ (ones col -> z row;
        gamma and biases folded into the weights)
      - ACT exp evicts PSUM->SBUF bf16; (1-I) mask mult for H family
      - PV matmul  [65, 128] = vT~^T P^T ; row 64 accumulates z
  * PV tiles evict into out_bf [65, 16384]; row 64 assembles z
  * rz = 1/z via chop->reciprocal_approx_fast->replicate (DMA)
  * out = out_bf * rz + x  (x re-read in f32), DMA to HBM
"""

import time

import numpy as np

import jax

import concourse.bass as bass
import concourse.bacc as bacc
import concourse.tile as tile
from concourse import mybir
from concourse.bass2jax import (
    _bass_exec_p,
    install_neuronx_cc_hook,
    partition_id_tensor,
)
from concourse.masks import make_identity

B, C, H, W = 16, 64, 128, 128
HW = H * W
C8 = 8
N_CORES = 8
B2 = B // N_CORES  # batches per core
CA = C + 1  # channel dim augmented with the bias/ones row
F32 = mybir.dt.float32
BF16 = mybir.dt.bfloat16
AF = mybir.ActivationFunctionType
ALU = mybir.AluOpType


def build_nc():
    nc = bacc.Bacc(
        "TRN2", target_bir_lowering=False, debug=False, enable_asserts=True
    )
    x_d = nc.dram_tensor("x", [B2, C, HW], F32, kind="ExternalInput")
    wq_d = nc.dram_tensor("wq", [C8, C], F32, kind="ExternalInput")
    bq_d = nc.dram_tensor("bq", [C8, 1], F32, kind="ExternalInput")
    wk_d = nc.dram_tensor("wk", [C8, C], F32, kind="ExternalInput")
    bk_d = nc.dram_tensor("bk", [C8, 1], F32, kind="ExternalInput")
    wv_d = nc.dram_tensor("wv", [C, C], F32, kind="ExternalInput")
    bv_d = nc.dram_tensor("bv", [C, 1], F32, kind="ExternalInput")
    g_d = nc.dram_tensor("gamma", [1, 1], F32, kind="ExternalInput")
    out_d = nc.dram_tensor("out", [B2, C, HW], F32, kind="ExternalOutput")

    with tile.TileContext(nc) as tc:
        _build_body(nc, tc, x_d, wq_d, bq_d, wk_d, bk_d, wv_d, bv_d, g_d, out_d)
    nc.compile()
    return nc


def _build_body(nc, tc, x_d, wq_d, bq_d, wk_d, bk_d, wv_d, bv_d, g_d, out_d):
    from contextlib import ExitStack

    with ExitStack() as ctx:
        ep = ctx.enter_context  # shorthand

        # ---- persistent SBUF state (bufs=1 pools) ----
        const = ep(tc.tile_pool(name="const", bufs=1))
        big = ep(tc.tile_pool(name="big", bufs=1))

        xaug = big.tile([CA, HW], BF16, tag="xaug")
        gq = big.tile([CA, HW], BF16, tag="gq")
        vt_col = big.tile([128, CA * W], BF16, tag="vtc")  # tile w at [:, 65w:65w+65]
        vt_row = big.tile([128, CA * H], BF16, tag="vtr")
        out_bf = big.tile([CA, HW], BF16, tag="outbf")  # rows 0..63 att, row 64 z

        gt_bf = const.tile([CA, CA], BF16, tag="gt")  # (Wk'^T Wq')^T
        wvtg = const.tile([CA, CA], BF16, tag="wvtg")  # [gamma*Wv^T~ | e64]
        mask = const.tile([128, 128], BF16, tag="mask")  # 1 - I
        ident = const.tile([C, C], BF16, tag="ident")
        z_sq = const.tile([128, 128], BF16, tag="zsq")
        z_f32 = const.tile([128, 128], F32, tag="zf32")
        rz_f32 = const.tile([128, 128], F32, tag="rzf32")
        rz_sq = const.tile([128, 128], BF16, tag="rzsq")

        # ---- scratch pools ----
        wpool = ep(tc.tile_pool(name="wts", bufs=1))
        chunks = ep(tc.tile_pool(name="chunks", bufs=2))
        fchunks = ep(tc.tile_pool(name="fchunks", bufs=3))
        ppool = ep(tc.tile_pool(name="ppool", bufs=6))
        dram = ep(tc.tile_pool(name="dram", bufs=1, space="DRAM"))
        rz_dram = dram.tile([1, HW], BF16, tag="rzd")
        z_dram = dram.tile([1, HW], BF16, tag="zd")
        ps_small = ep(tc.tile_pool(name="ps_small", bufs=3, space="PSUM"))
        ps_t = ep(tc.tile_pool(name="ps_t", bufs=1, space="PSUM"))
        ps_e = ep(tc.tile_pool(name="ps_e", bufs=2, space="PSUM"))

        # =================== setup ===================
        # raw weights to SBUF (f32)
        wq_f = wpool.tile([C8, C], F32, tag="wqf")
        wk_f = wpool.tile([C8, C], F32, tag="wkf")
        wv_f = wpool.tile([C, C], F32, tag="wvf")
        bq_f = wpool.tile([C8, 1], F32, tag="bqf")
        bk_f = wpool.tile([C8, 1], F32, tag="bkf")
        bv_f = wpool.tile([C, 1], F32, tag="bvf")
        g_f = wpool.tile([1, 1], F32, tag="gf")
        nc.sync.dma_start(wq_f[:], wq_d.ap())
        nc.sync.dma_start(wk_f[:], wk_d.ap())
        nc.sync.dma_start(wv_f[:], wv_d.ap())
        nc.sync.dma_start(bq_f[:], bq_d.ap())
        nc.sync.dma_start(bk_f[:], bk_d.ap())
        nc.sync.dma_start(bv_f[:], bv_d.ap())
        nc.sync.dma_start(g_f[:], g_d.ap())

        # augmented q/k weights  Wq' = [Wq | bq]  -> bf16
        wqp = wpool.tile([C8, CA], BF16, tag="wqp")
        wkp = wpool.tile([C8, CA], BF16, tag="wkp")
        nc.vector.tensor_copy(wqp[:, 0:C], wq_f[:])
        nc.vector.tensor_copy(wqp[:, C:CA], bq_f[:])
        nc.vector.tensor_copy(wkp[:, 0:C], wk_f[:])
        nc.vector.tensor_copy(wkp[:, C:CA], bk_f[:])

        # GT = Wq'^T Wk'  (so that GT^T @ x~ contracts correctly)
        gt_ps = ps_small.tile([CA, 512], F32, tag="mp")
        nc.tensor.matmul(gt_ps[:, 0:CA], wqp[:], wkp[:], start=True, stop=True)
        nc.scalar.copy(gt_bf[:], gt_ps[:, 0:CA])

        # identity (bf16) for PE transposes
        make_identity(nc, ident[:])

        # gamma broadcast to [C, 1] f32 via f32 matmul with ones row
        ones_row = wpool.tile([1, C], F32, tag="ones_row")
        nc.vector.memset(ones_row[:], 1.0)
        gcol_ps = ps_small.tile([C, 512], F32, tag="mp")
        nc.tensor.matmul(gcol_ps[:, 0:1], ones_row[:], g_f[:], start=True, stop=True)
        gcol = wpool.tile([C, 1], F32, tag="gcol")
        nc.scalar.copy(gcol[:], gcol_ps[:, 0:1])

        # WvT~g: rows 0..63 = gamma*Wv^T, row 64 = gamma*bv^T, col 64 = e_64
        wv_bf = wpool.tile([C, C], BF16, tag="wvbf")
        nc.vector.tensor_copy(wv_bf[:], wv_f[:])
        wvt_ps = ps_t.tile([C, 512], BF16, tag="mpT")
        nc.tensor.matmul(
            wvt_ps[:, 0:C], wv_bf[:], ident[:], is_transpose=True, start=True, stop=True
        )
        nc.scalar.activation(
            wvtg[0:C, 0:C], wvt_ps[:, 0:C], AF.Copy, scale=gcol[:]
        )
        bv_bf = wpool.tile([C, 1], BF16, tag="bvbf")
        nc.vector.tensor_copy(bv_bf[:], bv_f[:])
        bvt_ps = ps_t.tile([1, 512], BF16, tag="mpT")
        nc.tensor.matmul(
            bvt_ps[:, 0:C],
            bv_bf[:],
            ident[:],
            is_transpose=True,
            start=True,
            stop=True,
        )
        g11 = wpool.tile([1, 1], F32, tag="g11")
        nc.vector.tensor_copy(g11[:], g_f[:])
        nc.scalar.activation(wvtg[C : C + 1, 0:C], bvt_ps[:, 0:C], AF.Copy, scale=g11[:])
        nc.vector.memset(wvtg[0:C, C : C + 1], 0.0)
        nc.vector.memset(wvtg[C : C + 1, C : C + 1], 1.0)

        # mask = 1 - I  (bf16)
        nc.vector.memset(mask[:], 1.0)
        nc.gpsimd.affine_select(
            out=mask[:],
            in_=mask[:],
            compare_op=ALU.not_equal,
            fill=0.0,
            base=0,
            pattern=[[-1, 128]],
            channel_multiplier=1,
        )

        # ones row of x~ (written once; rows 0..63 rewritten per batch)
        nc.vector.memset(xaug[C : C + 1, :], 1.0)

        # =================== per-batch body ===================
        for b in range(B2):
            xa3 = xaug[0:C, :].rearrange("c (h w) -> c h w", w=W)
            ga3 = gq[:, :].rearrange("c (h w) -> c h w", w=W)
            xf3 = xaug[:, :].rearrange("c (h w) -> c h w", w=W)

            # ---- load x, cast to bf16 into xaug rows 0..63 ----
            NL = 2048
            for k in range(HW // NL):
                xc = chunks.tile([C, NL], F32, tag="xc")
                nc.sync.dma_start(xc[:], x_d.ap()[b, :, bass.ts(k, NL)])
                nc.vector.tensor_copy(xaug[0:C, bass.ts(k, NL)], xc[:])

            # ---- Gq = GT^T @ x~ ----
            for k in range(HW // 512):
                gq_ps = ps_small.tile([CA, 512], F32, tag="mp")
                nc.tensor.matmul(
                    gq_ps[:], gt_bf[:], xaug[:, bass.ts(k, 512)], start=True, stop=True
                )
                nc.scalar.copy(gq[:, bass.ts(k, 512)], gq_ps[:])

            # ---- vT tiles (7 per PSUM bank), both families ----
            for fam in range(2):
                vt_sb = vt_col if fam == 0 else vt_row
                for g0 in range(0, 128, 7):
                    gn = min(7, 128 - g0)
                    vt_ps = ps_small.tile([128, 512], F32, tag="mp")
                    for j in range(gn):
                        t = g0 + j
                        xs = xf3[:, :, t] if fam == 0 else xf3[:, t, :]
                        nc.tensor.matmul(
                            vt_ps[:, j * CA : (j + 1) * CA],
                            xs,
                            wvtg[:],
                            start=True,
                            stop=True,
                        )
                    ev = vt_sb[:, g0 * CA : (g0 + gn) * CA]
                    if (g0 // 7) % 2 == (0 if fam == 0 else 1):
                        nc.scalar.copy(ev, vt_ps[:, 0 : gn * CA])
                    else:
                        nc.vector.tensor_copy(ev, vt_ps[:, 0 : gn * CA])

            # ---- main loop: H family (copy) then W family (add) ----
            EG = 8  # energy tiles per PSUM chunk (2 banks)
            PG = 4  # PV tiles per PSUM bank
            for fam in (0, 1):
                vt_sb = vt_col if fam == 0 else vt_row
                for t0 in range(0, 128, EG):
                    e_ps = ps_e.tile([128, EG * 128], F32, tag="eps")
                    for j in range(EG):
                        t = t0 + j
                        xs = xf3[:, :, t] if fam == 0 else xf3[:, t, :]
                        gs = ga3[:, :, t] if fam == 0 else ga3[:, t, :]
                        nc.tensor.matmul(
                            e_ps[:, bass.ts(j, 128)], xs, gs, start=True, stop=True
                        )
                    p_sb = ppool.tile([128, EG * 128], BF16, tag="p")
                    nc.scalar.activation(p_sb[:], e_ps[:], AF.Exp)
                    if fam == 0:
                        # zero the diagonal of each tile: P *= (1 - I)
                        m_b = mask[:].unsqueeze(1).broadcast_to((128, EG, 128))
                        p3 = p_sb[:].rearrange("p (t q) -> p t q", t=EG)
                        nc.vector.tensor_tensor(p3, p3, m_b, op=ALU.mult)
                    for q0 in range(0, EG, PG):
                        pv_ps = ps_small.tile([CA, 512], F32, tag="mp")
                        for j in range(PG):
                            t = t0 + q0 + j
                            nc.tensor.matmul(
                                pv_ps[:, bass.ts(j, 128)],
                                vt_sb[:, t * CA : (t + 1) * CA],
                                p_sb[:, bass.ts(q0 + j, 128)],
                                start=True,
                                stop=True,
                            )
                        if fam == 0:
                            # H family: scatter-copy to out_bf[:, h*W + w]
                            w0 = t0 + q0
                            ob = out_bf[:].rearrange("c (h w) -> c h w", w=W)[
                                :, :, w0 : w0 + PG
                            ]
                            pv3 = pv_ps[:].rearrange("c (t h) -> c h t", t=PG)
                            if (w0 // PG) % 2 == 0:
                                nc.scalar.copy(ob, pv3)
                            else:
                                nc.vector.tensor_copy(ob, pv3)
                        else:
                            # W family: contiguous add into out_bf
                            dst = out_bf[:, (t0 + q0) * W : (t0 + q0 + PG) * W]
                            nc.vector.tensor_tensor(
                                dst, pv_ps[:], dst, op=ALU.add
                            )

            # ---- rz = 1/z  (z row -> DRAM -> square tile -> recip -> DRAM) ----
            nc.sync.dma_start(z_dram[:], out_bf[C : C + 1, :])
            nc.sync.dma_start(
                z_sq[:], z_dram[:].rearrange("o (p q) -> (o p) q", q=128)
            )
            nc.vector.tensor_copy(z_f32[:], z_sq[:])
            nc.vector.reciprocal_approx_fast(rz_f32[:], z_f32[:])
            nc.vector.tensor_copy(rz_sq[:], rz_f32[:])
            nc.sync.dma_start(
                rz_dram[:].rearrange("o (p q) -> (o p) q", q=128), rz_sq[:]
            )

            # ---- final: out = att*rz + x ----
            NF = 1024
            for k in range(HW // NF):
                rzb = fchunks.tile([C, NF], BF16, tag="rzb")
                nc.sync.dma_start(
                    rzb[:], rz_dram[:, bass.ts(k, NF)].broadcast_to((C, NF))
                )
                tb = fchunks.tile([C, NF], BF16, tag="tb")
                nc.vector.tensor_tensor(
                    tb[:], out_bf[0:C, bass.ts(k, NF)], rzb[:], op=ALU.mult
                )
                xf = fchunks.tile([C, NF], F32, tag="xf")
                nc.sync.dma_start(xf[:], x_d.ap()[b, :, bass.ts(k, NF)])
                of = fchunks.tile([C, NF], F32, tag="of")
                nc.gpsimd.tensor_tensor(of[:], xf[:], tb[:], op=ALU.add)
                nc.sync.dma_start(out_d.ap()[b, :, bass.ts(k, NF)], of[:])


# =====================================================================
# host-side wrapper: shard inputs, run the Bass program on 8 cores
# =====================================================================

_cache = {}


def _get_runner():
    if "run" in _cache:
        return _cache["run"]
    from jax.sharding import Mesh, PartitionSpec, NamedSharding
    from jax.experimental.shard_map import shard_map

    nc = build_nc()
    install_neuronx_cc_hook()

    in_names = ["x", "wq", "bq", "wk", "bk", "wv", "bv", "gamma", "out",
                "partition_id"]
    out_names = ["out"]
    out_avals = [jax.core.ShapedArray((B2, C, HW), np.float32)]

    def _body(*args):
        outs = _bass_exec_p.bind(
            *args,
            partition_id_tensor(),
            out_avals=tuple(out_avals),
            in_names=tuple(in_names),
            out_names=tuple(out_names),
            lowering_input_output_aliases=(),
            sim_require_finite=True,
            sim_require_nnan=True,
            nc=nc,
        )
        return outs[0]

    devices = jax.devices()[:N_CORES]
    mesh = Mesh(np.asarray(devices), ("core",))
    spec = NamedSharding(mesh, PartitionSpec("core"))
    P = PartitionSpec("core")
    f = jax.jit(
        shard_map(
            _body,
            mesh=mesh,
            in_specs=(P,) * 9,
            out_specs=P,
            check_rep=False,
        ),
        keep_unused=True,
    )
    _cache["run"] = (f, mesh, spec)
    return _cache["run"]


def _shard_inputs(x, Wq, bq, Wk, bk, Wv, bv, gamma):
    """Concatenated per-core operands (core-major on axis 0)."""
    xs = np.ascontiguousarray(x, np.float32).reshape(N_CORES, B2, C, HW)
    xs = xs.reshape(N_CORES * B2, C, HW)

    def rep(a, shape):
        a = np.ascontiguousarray(a, np.float32).reshape(shape)
        return np.concatenate([a] * N_CORES, axis=0)

    return [
        xs,
        rep(Wq, (C8, C)),
        rep(np.asarray(bq).reshape(C8, 1), (C8, 1)),
        rep(Wk, (C8, C)),
        rep(np.asarray(bk).reshape(C8, 1), (C8, 1)),
        rep(Wv, (C, C)),
        rep(np.asarray(bv).reshape(C, 1), (C, 1)),
        rep(np.asarray(gamma).reshape(1, 1), (1, 1)),
        np.zeros((N_CORES * B2, C, HW), np.float32),
    ]


def kernel(x, Wq, bq, Wk, bk, Wv, bv, gamma):
    f, mesh, spec = _get_runner()
    ops = _shard_inputs(x, Wq, bq, Wk, bk, Wv, bv, gamma)
    out = f(*ops)
    out = np.asarray(out).reshape(B, C, H, W).astype(np.float32)
    return out
